# revision 30
# baseline (speedup 1.0000x reference)
"""Trainium2 Bass kernel for nn_DensePoseV1ConvXGNInsHead:
2x (conv3x3 64->64 -> per-instance BN -> ReLU) on [8,64,256,256],
data-parallel one image per NeuronCore across 8 cores.

Structure (per core / image; A = rows 0:128 on partitions 0:64,
B = rows 128:256 on partitions 64:128):
 - conv3x3 as 9 shifted fp16 matmuls per 2-row chunk, block-diagonal
   [A|B] 128-partition weights, PSUM accumulation.
 - per-(image,instance) BN stats via PE transposes + fp8e4 DoubleRow mask
   matmuls (pair = A/B half) accumulating [18, s1|s2] in one PSUM bank;
   finalize entirely on partitions 0:18 (background handled by a host-side
   sqrt(eps) gamma row and zero inv-count).
 - normalize: tn = y + mprimeE accumulated on the PE (mask-expansion matmul
   + identity matmul into one PSUM bank); out = relu(tn) * aaE with Act
   relu + DVE multiply (gamma > 0 assumed, true for BN in this model).

Self-contained: only imports the system concourse stack from /opt/trn_rl_repo.
"""
import os
import sys
import types

sys.path.insert(0, "/opt/trn_rl_repo")

import numpy as np

import concourse.bass as bass
import concourse.tile as tile
from concourse import mybir
from concourse.vector_clock import ScopedClock

f16 = mybir.dt.float16
f32 = mybir.dt.float32
f8 = mybir.dt.float8e4
ALU = mybir.AluOpType
ACT = mybir.ActivationFunctionType
DRM = mybir.MatmulPerfMode.DoubleRow

C = 64          # channels
W = 256         # image width
PITCH = 272     # padded row pitch (16 left pad + 256 data; borrows next pad)
LP = 16         # left pad elements
R = 4           # conv rows per block (per half)
GS = 4          # norm groups per output store tile (8 rows)
EPS = 1e-5

# ---------------------------------------------------------------------------
# walrus workaround: split the Tile exit-drain's sem waits (installed walrus
# rejects instructions with >2 sync waits)
# ---------------------------------------------------------------------------
_patched = False


def _install_tile_patch():
    global _patched
    if _patched:
        return
    _patched = True

    def _drain_and_barrier(self, tick_clock, wait_clock):
        nc = self.nc
        drain_inst = nc.sync.drain()
        wait_clock.add_sem_waits(
            drain_inst.ins, ScopedClock({None: tick_clock.global_clock})
        )
        si = drain_inst.ins.sync_info
        waits = list(si.on_wait or [])
        if len(waits) > 1:
            si.on_wait = waits[:1]
            for i in range(1, len(waits)):
                nop = nc.sync.nop()
                nop.ins.sync_info = mybir.SyncInfo(
                    on_wait=waits[i : i + 1], on_update=[]
                )
        nc.all_engine_barrier()
        popped = nc._tile_sem_poison_stack.pop()
        assert popped is self._sem_poison
        nc.clear_and_free_semaphores(list(self.sems.allocated().values()))
        nc.all_engine_barrier()

    tile.TileContext._drain_and_barrier = _drain_and_barrier


# ---------------------------------------------------------------------------
# NTFF profiling shim (antenv.axon_hooks is absent in this image)
# ---------------------------------------------------------------------------
def _install_ntff_shim():
    if "antenv.axon_hooks" in sys.modules:
        return
    mod = types.ModuleType("antenv.axon_hooks")
    state = {"hook": None}
    mod.set_axon_ntff_profile_hook = lambda h: state.__setitem__("hook", h)
    mod.get_axon_ntff_profile_hook = lambda: state["hook"]
    sys.modules["antenv.axon_hooks"] = mod
    try:
        import antenv

        antenv.axon_hooks = mod
    except ImportError:
        pass
    try:
        from trn_agent_boot.trn_boot import _ntff_profile_via_ctypes

        h = _ntff_profile_via_ctypes("/opt/axon/libaxon_pjrt.so")
        mod.set_axon_ntff_profile_hook(h)
    except Exception:
        pass


def yoff(slot):
    return slot * PITCH + LP


def _ap(base_ap, offset_elems, dims):
    """Build a sub-AP of base_ap at +offset (elements), with given free dims."""
    return bass.AP(
        tensor=base_ap.tensor,
        offset=base_ap.offset + offset_elems,
        ap=[base_ap.ap[0]] + dims,
    )


def emit(nc, H):
    """Emit the full 2-layer kernel for an HxW image (H=256 in production)."""
    HH = H // 2
    NB = HH // R            # conv blocks per layer
    NCI = HH * 2            # 128-px chunk pairs (A+B) per layer
    HW = H * W
    HW2 = HH * W
    assert HH % R == 0 and (HH // 2) % GS == 0

    xh = nc.declare_dram_parameter("xh", [C, HW], f16, isOutput=False)
    idsf = nc.declare_dram_parameter("idsf", [HW], f16, isOutput=False)
    rcnt = nc.declare_dram_parameter("rcnt", [18, 1], f32, isOutput=False)
    kvec = nc.declare_dram_parameter("kvec", [18, 1], f32, isOutput=False)
    w0d = nc.declare_dram_parameter("w0d", [128, 9, 128], f16, isOutput=False)
    w1d = nc.declare_dram_parameter("w1d", [128, 9, 128], f16, isOutput=False)
    id128 = nc.declare_dram_parameter("id128", [128, 128], f16, isOutput=False)
    g18a = nc.declare_dram_parameter("g18a", [18, C], f32, isOutput=False)
    b18a = nc.declare_dram_parameter("b18a", [18, C], f32, isOutput=False)
    g18b = nc.declare_dram_parameter("g18b", [18, C], f32, isOutput=False)
    b18b = nc.declare_dram_parameter("b18b", [18, C], f32, isOutput=False)
    bdm = nc.declare_dram_parameter("bdm", [18, 128], f16, isOutput=False)
    out = nc.declare_dram_parameter("out", [C, HW], f16, isOutput=True)

    with tile.TileContext(nc) as tc:
        import contextlib

        with contextlib.ExitStack() as ctx:
            const = ctx.enter_context(tc.tile_pool(name="const", bufs=1))
            xbp = ctx.enter_context(tc.tile_pool(name="xbp", bufs=1))
            stripp = ctx.enter_context(tc.tile_pool(name="stripp", bufs=3))
            normp = ctx.enter_context(tc.tile_pool(name="normp", bufs=3))
            outp = ctx.enter_context(tc.tile_pool(name="outp", bufs=2))
            smallp = ctx.enter_context(tc.tile_pool(name="smallp", bufs=2))
            idsmp = ctx.enter_context(tc.tile_pool(name="idsmp", bufs=2))
            psc = ctx.enter_context(tc.tile_pool(name="psc", bufs=5, space="PSUM"))
            ptp = ctx.enter_context(tc.tile_pool(name="ptp", bufs=2, space="PSUM"))
            pss = ctx.enter_context(tc.tile_pool(name="pss", bufs=1, space="PSUM"))

            # ---- persistent y buffer (pitched, slots 0..HH+1 per half)
            ysb = const.tile([128, (HH + 2) * PITCH + LP], f16)
            # zero: all left pads (incl. trailing pad), top halo A, bottom halo B
            nc.vector.memset(_ap(ysb[:], 0, [[PITCH, HH + 3], [1, LP]]), 0.0)
            nc.vector.memset(_ap(ysb[0:64, :], yoff(0), [[1, W]]), 0.0)
            nc.vector.memset(_ap(ysb[64:128, :], yoff(HH + 1), [[1, W]]), 0.0)

            xbs = []
            for i in range(4):
                xb = xbp.tile([128, (R + 2) * PITCH + LP], f16, tag=f"xb{i}")
                nc.vector.memset(_ap(xb[:], 0, [[PITCH, R + 3], [1, LP]]), 0.0)
                xbs.append(xb)

            def xb_load(b, eng=None):
                eng = eng or nc.sync
                r0 = b * R
                xb = xbs[b % 4]
                if b == 0:
                    nc.vector.memset(
                        _ap(xb[0:64, :], yoff(0), [[1, W]]), 0.0
                    )
                if b == NB - 1:
                    nc.vector.memset(
                        _ap(xb[64:128, :], yoff(R + 1), [[1, W]]), 0.0
                    )
                lo_a = r0 - 1
                s_a = 0
                if b == 0:
                    lo_a, s_a = 0, 1
                n_a = r0 + R - lo_a + 1
                eng.dma_start(
                    out=_ap(xb[0:64, :], yoff(s_a), [[PITCH, n_a], [1, W]]),
                    in_=bass.AP(
                        tensor=xh[:].tensor,
                        offset=lo_a * W,
                        ap=[[HW, 64], [W, n_a], [1, W]],
                    ),
                )
                hb_lo = HH + r0 - 1
                n_b = R + 2 if b < NB - 1 else R + 1
                eng.dma_start(
                    out=_ap(xb[64:128, :], yoff(0), [[PITCH, n_b], [1, W]]),
                    in_=bass.AP(
                        tensor=xh[:].tensor,
                        offset=hb_lo * W,
                        ap=[[HW, 64], [W, n_b], [1, W]],
                    ),
                )

            # first conv inputs + layer-0 weights first on the DMA queue;
            # w0 is loaded per-tap so block 0's matmuls start after tap 0
            wts = []
            for wd in (w0d, w1d):
                wt = const.tile([128, 9, 128], f16, tag="wt")
                wts.append(wt)
            xb_load(0)
            nc.sync.dma_start(out=wts[0][:, 0:2, :], in_=w0d[:, 0:2, :])
            if NB > 1:
                xb_load(1)
            nc.sync.dma_start(out=wts[0][:, 2:9, :], in_=w0d[:, 2:9, :])

            # ---- small constants
            id128sb = const.tile([128, 128], f16)
            nc.sync.dma_start(out=id128sb[:], in_=id128[:])
            rcsb = const.tile([18, 1], f32)
            nc.sync.dma_start(out=rcsb[:], in_=rcnt[:])
            kvecsb = const.tile([18, 1], f32)
            nc.sync.dma_start(out=kvecsb[:], in_=kvec[:])
            epsap = const.tile([18, 1], f32)
            nc.vector.memset(epsap[:], EPS)
            ktile = const.tile([128, 9], f16)
            nc.gpsimd.iota(
                ktile[:], pattern=[[1, 9]], base=0, channel_multiplier=0,
                allow_small_or_imprecise_dtypes=True,
            )
            nc.vector.memset(ktile[:, 8:9], -1.0)
            bdmsb = const.tile([18, 128], f16)
            nc.sync.dma_start(out=bdmsb[:], in_=bdm[:])
            gam = []
            bet = []
            for gg, bb in ((g18a, b18a), (g18b, b18b)):
                gt = const.tile([18, C], f32, tag="gam")
                bt = const.tile([18, C], f32, tag="bet")
                nc.sync.dma_start(out=gt[:], in_=gg[:])
                nc.sync.dma_start(out=bt[:], in_=bb[:])
                gam.append(gt)
                bet.append(bt)
            nc.sync.dma_start(out=wts[1][:], in_=w1d[:])

            # ---- ids: pixel-major [128 px, global chunks] via PE
            # transpose. The load is issued here (DMA queue position) but
            # the PE transposes are emitted after conv block 0 so they do
            # not gate the in-order PE's start on the idsq DMA.
            F = HW // 128   # elements per partition in the contiguous load
            idp2 = const.tile([128, HW // 128], f16)
            maskpm = const.tile([128, NCI, 64], f8)
            if F % 128 == 0:
                idsq = idsmp.tile([128, F], f16, tag="idsq")
                nc.sync.dma_start(
                    out=idsq[:],
                    in_=bass.AP(tensor=idsf[:].tensor, offset=0,
                                ap=[[F, 128], [1, F]]),
                )
            else:
                nc.sync.dma_start(
                    out=idp2[:],
                    in_=bass.AP(tensor=idsf[:].tensor, offset=0,
                                ap=[[1, 128], [128, F]]),
                )

            def build_masks():
                if F % 128 == 0:
                    KT = F // 128
                    for k in range(KT):
                        ptsI = psc.tile([128, 128], f16, tag="cps",
                                        name=f"idT{k}")
                        nc.tensor.transpose(
                            ptsI[:], idsq[:, 128 * k : 128 * (k + 1)],
                            id128sb[:]
                        )
                        nc.vector.tensor_copy(
                            _ap(idp2[:], k, [[KT, 128]]), ptsI[:]
                        )
                # pixel-major one-hot masks, f8, duplicated per half, padded
                # so the DoubleRow stats lhsT is [[32,2],[1,18]] (16B pair
                # stride): per ci, 64 cols: A-dup18 at 0, B-dup18 at 32
                for h in (0, 1):
                    for d in (0, 1):
                        nc.vector.tensor_tensor(
                            _ap(maskpm[:], 32 * h + 9 * d,
                                [[64, NCI], [1, 9]]),
                            _ap(idp2[:], h * NCI, [[1, NCI], [0, 9]]),
                            _ap(ktile[:], 0, [[0, NCI], [1, 9]]),
                            ALU.is_equal,
                        )

            # segment-major one-hot masks, f8, padded to 128 partitions
            # (zeros; small-partition matmul inputs stream slowly on HW)
            ms2 = const.tile([128, HW2], f8)
            nc.gpsimd.memset(ms2[:], 0.0)
            MCH = min(2048, HW2)
            for mc in range(HW2 // MCH):
                idsm = idsmp.tile([18, MCH], f16, tag="idsm", name=f"idsm{mc}")
                nc.sync.dma_start(
                    out=idsm[:],
                    in_=bass.AP(
                        tensor=idsf[:].tensor,
                        offset=mc * MCH,
                        ap=[[HW2, 2], [0, 9], [1, MCH]],
                    ),
                )
                nc.vector.tensor_scalar(
                    out=ms2[0:18, mc * MCH : (mc + 1) * MCH], in0=idsm[:],
                    scalar1=kvecsb[:], scalar2=None, op0=ALU.is_equal,
                )

            # block-diagonal f16 lhsT tiles for the expansion matmuls;
            # rows 18:128 stay zero (contraction padded to 128, matching
            # ms2). Allocated/memset up-front, off the finalize chain.
            ab_tiles = {}
            for LL in (0, 1):
                a_s = smallp.tile([128, 128], f16, tag="ab2s")
                a_o = smallp.tile([128, 128], f16, tag="ab2o")
                nc.vector.memset(a_s[:], 0.0)
                nc.vector.memset(a_o[:], 0.0)
                ab_tiles[LL] = (a_s, a_o)

            SLOT0 = {0: 1, 1: 0}     # y row r lives at slot r+SLOT0[L]
            stats_t = {}
            strip_tiles = {0: {}, 1: {}}
            scnt = {0: 0, 1: 0}
            for LL in (0, 1):
                stats_t[LL] = pss.tile([18, 128], f32, tag="stats",
                                       name=f"stats{LL}")

            def conv_block(L, b, stash=False):
                wt = wts[L]
                slot0 = SLOT0[L]
                r0 = b * R
                if stash:
                    # L1 block 0 runs from the xb0 stash (its ysb input
                    # window is overwritten by block 1's output by now)
                    src_t = xbs[0]
                    loc = lambda rr, dy: (rr + 1 + dy)
                elif L == 0:
                    src_t = xbs[b % 4]
                    loc = lambda rr, dy: (rr - r0 + 1 + dy)  # slot in xb
                else:
                    src_t = ysb
                    loc = lambda rr, dy: (rr + dy + 1)       # y1 slot

                for cp in range(R // 2):
                    rr = r0 + 2 * cp
                    pt = psc.tile([128, 512], f32, tag="cps",
                                  name=f"c{L}_{b}_{cp}")
                    for t in range(9):
                        dy, dx = t // 3 - 1, t % 3 - 1
                        off = yoff(loc(rr, dy)) + dx
                        rhs = _ap(src_t[:], off, [[PITCH, 2], [1, W]])
                        nc.tensor.matmul(
                            pt[:], wt[:, t, :], rhs,
                            start=(t == 0), stop=(t == 8),
                        )
                    nc.scalar.copy(
                        out=_ap(ysb[:], yoff(rr + slot0), [[PITCH, 2], [1, W]]),
                        in_=pt[:],
                    )
                if L == 0 and not stash and b + 2 < NB:
                    xb_load(b + 2)

            def transp_block(L, b):
                slot0 = SLOT0[L]
                r0 = b * R
                pts2 = ptp.tile([128, 1024], f16, tag="tp", name=f"tp{L}_{b}")
                for j in range(2 * R):
                    rr = r0 + j // 2
                    cs = j % 2
                    src = _ap(ysb[:], yoff(rr + slot0) + cs * 128, [[1, 128]])
                    nc.tensor.transpose(
                        pts2[:, j * 128 : (j + 1) * 128], src, id128sb[:]
                    )
                # strip layout per chunk: [yA(64) y2A(64) yB(64) y2B(64)]
                sp = stripp.tile([128, 2 * R, 256], f8, tag="strip",
                                 name=f"sp{L}_{b}")
                strip_tiles[L][b] = sp
                nc.scalar.copy(
                    out=_ap(sp[:], 0, [[256, 2 * R], [1, 64]]),
                    in_=_ap(pts2[:], 0, [[128, 2 * R], [1, 64]]),
                )
                nc.scalar.copy(
                    out=_ap(sp[:], 128, [[256, 2 * R], [1, 64]]),
                    in_=_ap(pts2[:], 64, [[128, 2 * R], [1, 64]]),
                )
                nc.vector.tensor_tensor(
                    _ap(sp[:], 64, [[128, 4 * R], [1, 64]]),
                    _ap(sp[:], 0, [[128, 4 * R], [1, 64]]),
                    _ap(sp[:], 0, [[128, 4 * R], [1, 64]]),
                    ALU.mult,
                )

            def stats_block(L, b):
                sp = strip_tiles[L].pop(b)
                for j in range(2 * R):
                    ci = b * 2 * R + j
                    lhsT = _ap(maskpm[:], ci * 64, [[32, 2], [1, 18]])
                    rhs = _ap(sp[:], j * 256, [[128, 2], [1, 128]])
                    nc.tensor.matmul(
                        stats_t[L][:], lhsT, rhs,
                        start=(scnt[L] == 0),
                        stop=(scnt[L] == NCI - 1),
                        perf_mode=DRM,
                    )
                    scnt[L] += 1

            def finalize(L):
                stats = stats_t[L]
                mean = smallp.tile([18, C], f32, tag="mean")
                e2 = smallp.tile([18, C], f32, tag="e2")
                nc.vector.tensor_scalar_mul(out=mean[:], in0=stats[:, 0:64],
                                            scalar1=rcsb[:])
                nc.vector.tensor_scalar_mul(out=e2[:], in0=stats[:, 64:128],
                                            scalar1=rcsb[:])
                var = smallp.tile([18, C], f32, tag="var")
                # mean^2 on Act, in parallel with the DVE chain
                nc.scalar.activation(out=var[:], in_=mean[:], func=ACT.Square)
                nc.vector.tensor_tensor(var[:], e2[:], var[:], ALU.subtract)
                sd = smallp.tile([18, C], f32, tag="sd")
                nc.scalar.activation(out=sd[:], in_=var[:], func=ACT.Sqrt,
                                     bias=epsap[:], scale=1.0)
                rstd = smallp.tile([18, C], f32, tag="rstd")
                nc.vector.reciprocal(out=rstd[:], in_=sd[:])
                aa = smallp.tile([18, C], f32, tag="aa")
                nc.vector.tensor_tensor(aa[:], rstd[:], gam[L][:], ALU.mult)
                inv = smallp.tile([18, C], f32, tag="inv")
                nc.vector.reciprocal(out=inv[:], in_=aa[:])
                mprime = smallp.tile([18, C], f32, tag="mprime")
                nc.vector.tensor_tensor(mprime[:], bet[L][:], inv[:], ALU.mult)
                nc.vector.tensor_tensor(mprime[:], mprime[:], mean[:],
                                        ALU.subtract)
                ab2s, ab2o = ab_tiles[L]
                nc.vector.tensor_tensor(
                    ab2s[0:18, :], _ap(aa[:], 0, [[0, 2], [1, C]]), bdmsb[:],
                    ALU.mult,
                )
                nc.vector.tensor_tensor(
                    ab2o[0:18, :], _ap(mprime[:], 0, [[0, 2], [1, C]]), bdmsb[:],
                    ALU.mult,
                )
                return ab2s, ab2o

            STQ = {0: None}
            pend0 = {}

            def flush_mult0():
                if pend0:
                    g0, (tr0, sE0, yv0) = pend0.popitem()
                    nc.vector.tensor_tensor(yv0, tr0[:], sE0[:], ALU.mult)

            def norm_group(L, g, ab2s, ab2o):
                # out = relu(y + mprimeE) * aaE  (gamma>0). Layer 0 (fused
                # with L1 conv, PE-bound): DVE add + Act relu + DVE mult,
                # one-group software pipeline so the DVE never waits on Act.
                # Layer 1 (tail, Act/DVE-paced): y-add on the PE via an
                # identity matmul into the mprimeE PSUM bank.
                slot0 = SLOT0[L]
                yv = _ap(ysb[:], yoff(2 * g + slot0), [[PITCH, 2], [1, W]])
                win = ms2[:, 2 * g * W : (2 * g + 2) * W]
                tnp = psc.tile([128, 512], f32, tag="cps", name=f"tn{L}_{g}")
                sEp = psc.tile([128, 512], f32, tag="cps", name=f"sE{L}_{g}")
                if L == 0:
                    nc.tensor.matmul(tnp[:], ab2o[:], win, start=True, stop=True)
                    nc.tensor.matmul(sEp[:], ab2s[:], win, start=True, stop=True)
                    tn = normp.tile([128, 512], f16, tag="tn", name=f"tn0_{g}")
                    nc.vector.tensor_tensor(tn[:], yv, tnp[:], ALU.add)
                    tr = normp.tile([128, 512], f16, tag="tr", name=f"tr0_{g}")
                    nc.scalar.activation(out=tr[:], in_=tn[:], func=ACT.Relu)
                    flush_mult0()
                    pend0[g] = (tr, sEp, yv)
                    return
                nc.tensor.matmul(tnp[:], ab2o[:], win, start=True, stop=False)
                nc.tensor.matmul(tnp[:], id128sb[:], yv, start=False, stop=True)
                nc.tensor.matmul(sEp[:], ab2s[:], win, start=True, stop=True)
                tr = normp.tile([128, 512], f16, tag="tr", name=f"tr{L}_{g}")
                nc.scalar.activation(out=tr[:], in_=tnp[:], func=ACT.Relu)
                if True:
                    gl = g % GS
                    if gl == 0:
                        STQ[0] = outp.tile([128, GS * 512], f16, tag="st",
                                           name=f"st{g // GS}")
                    dst = STQ[0][:, gl * 512 : (gl + 1) * 512]
                nc.vector.tensor_tensor(dst, tr[:], sEp[:], ALU.mult)
                if L == 1 and g % GS == GS - 1:
                    st = STQ[0]
                    gb = g // GS
                    eng = nc.sync if gb % 2 == 0 else nc.scalar
                    eng.dma_start(
                        out=bass.AP(tensor=out[:].tensor,
                                    offset=gb * 2 * GS * W,
                                    ap=[[HW, 64], [1, 2 * GS * W]]),
                        in_=st[0:64, :],
                    )
                    eng2 = nc.scalar if gb % 2 == 0 else nc.sync
                    eng2.dma_start(
                        out=bass.AP(tensor=out[:].tensor,
                                    offset=HW2 + gb * 2 * GS * W,
                                    ap=[[HW, 64], [1, 2 * GS * W]]),
                        in_=st[64:128, :],
                    )

            # ================= layer 0: conv + stats =================
            conv_block(0, 0)
            build_masks()
            if NB > 1:
                conv_block(0, 1)
            transp_block(0, 0)
            for b in range(2, NB):
                conv_block(0, b)
                transp_block(0, b - 1)
                stats_block(0, b - 2)
            transp_block(0, NB - 1)
            if NB > 1:
                stats_block(0, NB - 2)
            stats_block(0, NB - 1)
            ab2s0, ab2o0 = finalize(0)

            # ===== fused: layer-0 normalize + layer-1 conv/stats =====
            # L1 conv block order [1..NB-1, 0]: block 0 needs the B-half
            # top halo (= normalized A row HH-1, ready only after the last
            # norm group), block NB-1 needs the A-half bottom halo (= B row
            # 0, ready after group 0).
            seq = []

            def push_l1(bb):
                conv_block(1, bb, stash=(bb == 0))
                seq.append(bb)
                if len(seq) >= 2:
                    transp_block(1, seq[-2])
                if len(seq) >= 3:
                    stats_block(1, seq[-3])

            norm_group(0, 0, ab2s0, ab2o0)
            for g in range(1, HH // 2):
                norm_group(0, g, ab2s0, ab2o0)
                if g == 1:
                    # A-half bottom halo: slot HH+1 <- normalized B row 0
                    # (slot 1; its multiply was emitted inside group 1)
                    nc.sync.dma_start(
                        out=_ap(ysb[0:64, :], yoff(HH + 1), [[1, W]]),
                        in_=_ap(ysb[64:128, :], yoff(1), [[1, W]]),
                    )
                if g == 3:
                    # stash L1-block-0's input window (y1n rows 0..4, both
                    # halves) into xb0 before block 1's output clobbers it
                    nc.vector.memset(_ap(xbs[0][0:64, :], yoff(0), [[1, W]]),
                                     0.0)
                    nc.scalar.copy(
                        out=_ap(xbs[0][:], yoff(1), [[PITCH, R + 1], [1, W]]),
                        in_=_ap(ysb[:], yoff(1), [[PITCH, R + 1], [1, W]]),
                    )
                if g >= 5 and g % 2 == 1:
                    # with the one-group multiply lag, mults 0..g-1 are
                    # emitted after group g; block bb needs mults <= 2bb+2
                    bb = (g - 3) // 2
                    if 1 <= bb <= NB - 2:
                        push_l1(bb)
            flush_mult0()
            if NB > 2:
                push_l1(NB - 1)
            # B-half top halo for the stashed block 0:
            # xb0 B slot 0 <- normalized A row HH-1 (slot HH)
            nc.sync.dma_start(
                out=_ap(xbs[0][64:128, :], yoff(0), [[1, W]]),
                in_=_ap(ysb[0:64, :], yoff(HH), [[1, W]]),
            )
            push_l1(0)
            if NB == 2:
                push_l1(1)
            # drain the transpose/stats pipeline tail
            transp_block(1, seq[-1])
            stats_block(1, seq[-2])
            stats_block(1, seq[-1])
            ab2s1, ab2o1 = finalize(1)

            # ================= layer 1 normalize + store =================
            for g in range(HH // 2):
                norm_group(1, g, ab2s1, ab2o1)

    return nc


MAXW = 1


def _split_multi_waits(nc):
    """The installed walrus rejects instructions with >MAXW sync waits; hoist
    excess waits onto preceding same-engine nops."""
    nsplit = 0
    for fn in nc.m.functions:
        for blk in fn.blocks:
            insts = list(blk.instructions)
            out = []
            for inst in insts:
                si = inst.sync_info
                waits = list(si.on_wait) if (si and si.on_wait) else []
                if len(waits) > MAXW:
                    for i in range(0, len(waits) - MAXW, MAXW):
                        nop = mybir.InstNoOp(
                            name=f"WSPLIT-{nsplit}", ins=[], outs=[]
                        )
                        nsplit += 1
                        nop.engine = inst.engine
                        nop.sync_info = mybir.SyncInfo(
                            on_wait=waits[i : i + MAXW], on_update=[]
                        )
                        out.append(nop)
                    si.on_wait = waits[len(waits) - MAXW :]
                out.append(inst)
            if len(out) != len(insts):
                while len(blk.instructions):
                    blk.instructions.pop()
                for inst in out:
                    blk.instructions.append(inst)
    return nsplit


def build_nc(H=256, split_waits=True):
    _install_tile_patch()
    nc = bass.Bass()
    emit(nc, H)
    if split_waits:
        n = _split_multi_waits(nc)
        if n:
            print(f"kernel: split {n} multi-wait instructions")
    return nc


# ---------------------------------------------------------------------------
# host-side input prep
# ---------------------------------------------------------------------------
def prep_core_inputs(x_img, ids_img, w0, g0v, b0v, w1, g1v, b1v, H=256):
    """x_img [C,H,W] f32, ids_img [H,W] int -> input map for one core."""
    seg = np.where(ids_img < 0, 8, ids_img).astype(np.int64)

    m = {}
    m["xh"] = np.ascontiguousarray(x_img.reshape(C, H * W).astype(np.float16))
    m["idsf"] = np.ascontiguousarray(ids_img.reshape(H * W).astype(np.float16))
    cnt = np.bincount(seg.reshape(-1), minlength=9)[:9]
    rc9 = (1.0 / np.maximum(cnt, 1)).astype(np.float32)
    rc9[8] = 0.0  # background: forces mean=var=0 -> rstd=1/sqrt(eps)
    rc = np.concatenate([rc9, rc9])
    m["rcnt"] = rc.reshape(18, 1).astype(np.float32)
    kv9 = np.array([0, 1, 2, 3, 4, 5, 6, 7, -1], np.float32)
    m["kvec"] = np.concatenate([kv9, kv9]).reshape(18, 1)

    for name, wmat in (("w0d", w0), ("w1d", w1)):
        wd = np.zeros((9, 128, 128), np.float16)
        for t in range(9):
            dy, dx = t // 3, t % 3
            lhsT = wmat[:, :, dy, dx].T.astype(np.float16)  # [cin, cout]
            wd[t, 0:64, 0:64] = lhsT
            wd[t, 64:128, 64:128] = lhsT
        m[name] = np.ascontiguousarray(wd.transpose(1, 0, 2))  # [ci, t, co]

    m["id128"] = np.eye(128, dtype=np.float16)
    bdmask = np.zeros((18, 128), np.float16)
    bdmask[0:9, 0:64] = 1.0
    bdmask[9:18, 64:128] = 1.0
    m["bdm"] = bdmask
    for nmg, nmb, gv, bv in (("g18a", "b18a", g0v, b0v), ("g18b", "b18b", g1v, b1v)):
        g9 = np.broadcast_to(np.asarray(gv, np.float32), (9, C)).copy()
        b9 = np.broadcast_to(np.asarray(bv, np.float32), (9, C)).copy()
        g9[8, :] = np.sqrt(EPS)   # background row: aa = rstd*sqrt(eps) = 1
        b9[8, :] = 0.0
        m[nmg] = np.concatenate([g9, g9], 0).astype(np.float32)
        m[nmb] = np.concatenate([b9, b9], 0).astype(np.float32)
    return m


LAST_RESULT = None


def kernel(features, ins_indices_batch, w0, g0, b0, w1, g1, b1):
    global LAST_RESULT
    _install_ntff_shim()
    from concourse.bass_utils import run_bass_kernel_spmd
    from concourse import bass2jax as _b2j
    import traceback as _tb

    _b2j.install_neuronx_cc_hook()
    import libneuronxla as _lnx

    if not getattr(_lnx, "_ant_dbg_wrapped", False):
        _orig = _lnx.neuronx_cc

        def _dbg(*a, **k):
            try:
                return _orig(*a, **k)
            except BaseException:
                _tb.print_exc()
                raise

        _lnx.neuronx_cc = _dbg
        _lnx._ant_dbg_wrapped = True

    x = np.asarray(features, np.float32)
    ids = np.asarray(ins_indices_batch).astype(np.int64)
    w0 = np.asarray(w0, np.float32)
    w1 = np.asarray(w1, np.float32)
    N = x.shape[0]
    H = x.shape[2]

    nc = build_nc(H)
    in_maps = [
        prep_core_inputs(x[i], ids[i], w0, g0, b0, w1, g1, b1, H) for i in range(N)
    ]
    trace = bool(int(os.environ.get("BASS_KERNEL_TRACE", "0")))
    res = run_bass_kernel_spmd(nc, in_maps, list(range(N)), trace=trace)
    LAST_RESULT = res
    outs = [
        np.asarray(res.results[i]["out"], np.float32).reshape(C, H, W)
        for i in range(N)
    ]
    return np.stack(outs, 0)


# revision 31
# speedup vs baseline: 1.1436x; 1.1436x over previous
"""Trainium2 Bass kernel for nn_DensePoseV1ConvXGNInsHead:
2x (conv3x3 64->64 -> per-instance BN -> ReLU) on [8,64,256,256],
data-parallel one image per NeuronCore across 8 cores.

Structure (per core / image; A = rows 0:128 on partitions 0:64,
B = rows 128:256 on partitions 64:128):
 - conv3x3 as 9 shifted fp16 matmuls per 2-row chunk, block-diagonal
   [A|B] 128-partition weights, PSUM accumulation.
 - per-(image,instance) BN stats via PE transposes + fp8e4 DoubleRow mask
   matmuls (pair = A/B half) accumulating [18, s1|s2] in one PSUM bank;
   finalize entirely on partitions 0:18 (background handled by a host-side
   sqrt(eps) gamma row and zero inv-count).
 - normalize: tn = y + mprimeE accumulated on the PE (mask-expansion matmul
   + identity matmul into one PSUM bank); out = relu(tn) * aaE with Act
   relu + DVE multiply (gamma > 0 assumed, true for BN in this model).

Self-contained: only imports the system concourse stack from /opt/trn_rl_repo.
"""
import os
import sys
import types

sys.path.insert(0, "/opt/trn_rl_repo")

import numpy as np

import concourse.bass as bass
import concourse.tile as tile
from concourse import mybir
from concourse.vector_clock import ScopedClock

f16 = mybir.dt.float16
f32 = mybir.dt.float32
f8 = mybir.dt.float8e4
ALU = mybir.AluOpType
ACT = mybir.ActivationFunctionType
DRM = mybir.MatmulPerfMode.DoubleRow

C = 64          # channels
W = 256         # image width
PITCH = 272     # padded row pitch (16 left pad + 256 data; borrows next pad)
LP = 16         # left pad elements
R = 4           # conv rows per block (per half)
GS = 4          # norm groups per output store tile (8 rows)
EPS = 1e-5

# ---------------------------------------------------------------------------
# walrus workaround: split the Tile exit-drain's sem waits (installed walrus
# rejects instructions with >2 sync waits)
# ---------------------------------------------------------------------------
_patched = False


def _install_tile_patch():
    global _patched
    if _patched:
        return
    _patched = True

    def _drain_and_barrier(self, tick_clock, wait_clock):
        nc = self.nc
        drain_inst = nc.sync.drain()
        wait_clock.add_sem_waits(
            drain_inst.ins, ScopedClock({None: tick_clock.global_clock})
        )
        si = drain_inst.ins.sync_info
        waits = list(si.on_wait or [])
        if len(waits) > 1:
            si.on_wait = waits[:1]
            for i in range(1, len(waits)):
                nop = nc.sync.nop()
                nop.ins.sync_info = mybir.SyncInfo(
                    on_wait=waits[i : i + 1], on_update=[]
                )
        nc.all_engine_barrier()
        popped = nc._tile_sem_poison_stack.pop()
        assert popped is self._sem_poison
        nc.clear_and_free_semaphores(list(self.sems.allocated().values()))
        nc.all_engine_barrier()

    tile.TileContext._drain_and_barrier = _drain_and_barrier


# ---------------------------------------------------------------------------
# NTFF profiling shim (antenv.axon_hooks is absent in this image)
# ---------------------------------------------------------------------------
def _install_ntff_shim():
    if "antenv.axon_hooks" in sys.modules:
        return
    mod = types.ModuleType("antenv.axon_hooks")
    state = {"hook": None}
    mod.set_axon_ntff_profile_hook = lambda h: state.__setitem__("hook", h)
    mod.get_axon_ntff_profile_hook = lambda: state["hook"]
    sys.modules["antenv.axon_hooks"] = mod
    try:
        import antenv

        antenv.axon_hooks = mod
    except ImportError:
        pass
    try:
        from trn_agent_boot.trn_boot import _ntff_profile_via_ctypes

        h = _ntff_profile_via_ctypes("/opt/axon/libaxon_pjrt.so")
        mod.set_axon_ntff_profile_hook(h)
    except Exception:
        pass


def yoff(slot):
    return slot * PITCH + LP


def _ap(base_ap, offset_elems, dims):
    """Build a sub-AP of base_ap at +offset (elements), with given free dims."""
    return bass.AP(
        tensor=base_ap.tensor,
        offset=base_ap.offset + offset_elems,
        ap=[base_ap.ap[0]] + dims,
    )


def emit(nc, H):
    """Emit the full 2-layer kernel for an HxW image (H=256 in production)."""
    HH = H // 2
    NB = HH // R            # conv blocks per layer
    NCI = HH * 2            # 128-px chunk pairs (A+B) per layer
    HW = H * W
    HW2 = HH * W
    assert HH % R == 0 and (HH // 2) % GS == 0

    xh = nc.declare_dram_parameter("xh", [C, HW], f16, isOutput=False)
    idsf = nc.declare_dram_parameter("idsf", [HW], f16, isOutput=False)
    rcnt = nc.declare_dram_parameter("rcnt", [18, 1], f32, isOutput=False)
    kvec = nc.declare_dram_parameter("kvec", [18, 1], f32, isOutput=False)
    w0d = nc.declare_dram_parameter("w0d", [128, 9, 128], f16, isOutput=False)
    w1d = nc.declare_dram_parameter("w1d", [128, 9, 128], f16, isOutput=False)
    id128 = nc.declare_dram_parameter("id128", [128, 128], f16, isOutput=False)
    g18a = nc.declare_dram_parameter("g18a", [18, C], f32, isOutput=False)
    b18a = nc.declare_dram_parameter("b18a", [18, C], f32, isOutput=False)
    g18b = nc.declare_dram_parameter("g18b", [18, C], f32, isOutput=False)
    b18b = nc.declare_dram_parameter("b18b", [18, C], f32, isOutput=False)
    bdm = nc.declare_dram_parameter("bdm", [18, 128], f16, isOutput=False)
    out = nc.declare_dram_parameter("out", [C, HW], f16, isOutput=True)

    with tile.TileContext(nc) as tc:
        import contextlib

        with contextlib.ExitStack() as ctx:
            const = ctx.enter_context(tc.tile_pool(name="const", bufs=1))
            xbp = ctx.enter_context(tc.tile_pool(name="xbp", bufs=1))
            stripp = ctx.enter_context(tc.tile_pool(name="stripp", bufs=3))
            normp = ctx.enter_context(tc.tile_pool(name="normp", bufs=3))
            outp = ctx.enter_context(tc.tile_pool(name="outp", bufs=2))
            smallp = ctx.enter_context(tc.tile_pool(name="smallp", bufs=2))
            idsmp = ctx.enter_context(tc.tile_pool(name="idsmp", bufs=2))
            psc = ctx.enter_context(tc.tile_pool(name="psc", bufs=5, space="PSUM"))
            ptp = ctx.enter_context(tc.tile_pool(name="ptp", bufs=2, space="PSUM"))
            pss = ctx.enter_context(tc.tile_pool(name="pss", bufs=1, space="PSUM"))

            # ---- persistent y buffer (pitched, slots 0..HH+1 per half)
            ysb = const.tile([128, (HH + 2) * PITCH + LP], f16)
            # zero: all left pads (incl. trailing pad), top halo A, bottom halo B
            nc.vector.memset(_ap(ysb[:], 0, [[PITCH, HH + 3], [1, LP]]), 0.0)
            nc.vector.memset(_ap(ysb[0:64, :], yoff(0), [[1, W]]), 0.0)
            nc.vector.memset(_ap(ysb[64:128, :], yoff(HH + 1), [[1, W]]), 0.0)

            xbs = []
            for i in range(4):
                xb = xbp.tile([128, (R + 2) * PITCH + LP], f16, tag=f"xb{i}")
                nc.vector.memset(_ap(xb[:], 0, [[PITCH, R + 3], [1, LP]]), 0.0)
                xbs.append(xb)

            def xb_load(b, eng=None):
                eng = eng or nc.sync
                r0 = b * R
                xb = xbs[b % 4]
                if b == 0:
                    nc.vector.memset(
                        _ap(xb[0:64, :], yoff(0), [[1, W]]), 0.0
                    )
                if b == NB - 1:
                    nc.vector.memset(
                        _ap(xb[64:128, :], yoff(R + 1), [[1, W]]), 0.0
                    )
                lo_a = r0 - 1
                s_a = 0
                if b == 0:
                    lo_a, s_a = 0, 1
                n_a = r0 + R - lo_a + 1
                eng.dma_start(
                    out=_ap(xb[0:64, :], yoff(s_a), [[PITCH, n_a], [1, W]]),
                    in_=bass.AP(
                        tensor=xh[:].tensor,
                        offset=lo_a * W,
                        ap=[[HW, 64], [W, n_a], [1, W]],
                    ),
                )
                hb_lo = HH + r0 - 1
                n_b = R + 2 if b < NB - 1 else R + 1
                eng.dma_start(
                    out=_ap(xb[64:128, :], yoff(0), [[PITCH, n_b], [1, W]]),
                    in_=bass.AP(
                        tensor=xh[:].tensor,
                        offset=hb_lo * W,
                        ap=[[HW, 64], [W, n_b], [1, W]],
                    ),
                )

            # first conv inputs + layer-0 weights first on the DMA queue;
            # w0 is loaded per-tap so block 0's matmuls start after tap 0
            wts = []
            for wd in (w0d, w1d):
                wt = const.tile([128, 9, 128], f16, tag="wt")
                wts.append(wt)
            xb_load(0)
            nc.sync.dma_start(out=wts[0][:, 0:2, :], in_=w0d[:, 0:2, :])
            if NB > 1:
                xb_load(1)
            nc.sync.dma_start(out=wts[0][:, 2:9, :], in_=w0d[:, 2:9, :])

            # ---- small constants
            id128sb = const.tile([128, 128], f16)
            nc.sync.dma_start(out=id128sb[:], in_=id128[:])
            rcsb = const.tile([18, 1], f32)
            nc.sync.dma_start(out=rcsb[:], in_=rcnt[:])
            kvecsb = const.tile([18, 1], f32)
            nc.sync.dma_start(out=kvecsb[:], in_=kvec[:])
            epsap = const.tile([18, 1], f32)
            nc.vector.memset(epsap[:], EPS)
            ktile = const.tile([128, 9], f16)
            nc.gpsimd.iota(
                ktile[:], pattern=[[1, 9]], base=0, channel_multiplier=0,
                allow_small_or_imprecise_dtypes=True,
            )
            nc.vector.memset(ktile[:, 8:9], -1.0)
            bdmsb = const.tile([18, 128], f16)
            nc.sync.dma_start(out=bdmsb[:], in_=bdm[:])
            gam = []
            bet = []
            for gg, bb in ((g18a, b18a), (g18b, b18b)):
                gt = const.tile([18, C], f32, tag="gam")
                bt = const.tile([18, C], f32, tag="bet")
                nc.sync.dma_start(out=gt[:], in_=gg[:])
                nc.sync.dma_start(out=bt[:], in_=bb[:])
                gam.append(gt)
                bet.append(bt)
            nc.sync.dma_start(out=wts[1][:], in_=w1d[:])

            # ---- ids: pixel-major [128 px, global chunks] via PE
            # transpose. The load is issued here (DMA queue position) but
            # the PE transposes are emitted after conv block 0 so they do
            # not gate the in-order PE's start on the idsq DMA.
            F = HW // 128   # elements per partition in the contiguous load
            idp2 = const.tile([128, HW // 128], f16)
            maskpm = const.tile([128, NCI, 64], f8)
            if F % 128 == 0:
                idsq = idsmp.tile([128, F], f16, tag="idsq")
                nc.sync.dma_start(
                    out=idsq[:],
                    in_=bass.AP(tensor=idsf[:].tensor, offset=0,
                                ap=[[F, 128], [1, F]]),
                )
            else:
                nc.sync.dma_start(
                    out=idp2[:],
                    in_=bass.AP(tensor=idsf[:].tensor, offset=0,
                                ap=[[1, 128], [128, F]]),
                )

            def build_masks():
                if F % 128 == 0:
                    KT = F // 128
                    for k in range(KT):
                        ptsI = psc.tile([128, 128], f16, tag="cps",
                                        name=f"idT{k}")
                        nc.tensor.transpose(
                            ptsI[:], idsq[:, 128 * k : 128 * (k + 1)],
                            id128sb[:]
                        )
                        nc.vector.tensor_copy(
                            _ap(idp2[:], k, [[KT, 128]]), ptsI[:]
                        )
                # pixel-major one-hot masks, f8, duplicated per half, padded
                # so the DoubleRow stats lhsT is [[32,2],[1,18]] (16B pair
                # stride): per ci, 64 cols: A-dup18 at 0, B-dup18 at 32
                for h in (0, 1):
                    for d in (0, 1):
                        nc.vector.tensor_tensor(
                            _ap(maskpm[:], 32 * h + 9 * d,
                                [[64, NCI], [1, 9]]),
                            _ap(idp2[:], h * NCI, [[1, NCI], [0, 9]]),
                            _ap(ktile[:], 0, [[0, NCI], [1, 9]]),
                            ALU.is_equal,
                        )

            # segment-major one-hot masks, f8, padded to 128 partitions
            # (zeros; small-partition matmul inputs stream slowly on HW)
            ms2 = const.tile([128, HW2], f8)
            nc.gpsimd.memset(ms2[:], 0.0)
            MCH = min(2048, HW2)
            for mc in range(HW2 // MCH):
                idsm = idsmp.tile([18, MCH], f16, tag="idsm", name=f"idsm{mc}")
                nc.sync.dma_start(
                    out=idsm[:],
                    in_=bass.AP(
                        tensor=idsf[:].tensor,
                        offset=mc * MCH,
                        ap=[[HW2, 2], [0, 9], [1, MCH]],
                    ),
                )
                nc.vector.tensor_scalar(
                    out=ms2[0:18, mc * MCH : (mc + 1) * MCH], in0=idsm[:],
                    scalar1=kvecsb[:], scalar2=None, op0=ALU.is_equal,
                )

            # block-diagonal f16 lhsT tiles for the expansion matmuls;
            # rows 18:128 stay zero (contraction padded to 128, matching
            # ms2). Allocated/memset up-front, off the finalize chain.
            ab_tiles = {}
            for LL in (0, 1):
                a_s = smallp.tile([128, 128], f16, tag="ab2s")
                a_o = smallp.tile([128, 128], f16, tag="ab2o")
                nc.vector.memset(a_s[:], 0.0)
                nc.vector.memset(a_o[:], 0.0)
                ab_tiles[LL] = (a_s, a_o)

            SLOT0 = {0: 1, 1: 0}     # y row r lives at slot r+SLOT0[L]
            stats_t = {}
            strip_tiles = {0: {}, 1: {}}
            scnt = {0: 0, 1: 0}
            for LL in (0, 1):
                stats_t[LL] = pss.tile([18, 128], f32, tag="stats",
                                       name=f"stats{LL}")

            def conv_block(L, b, stash=False):
                wt = wts[L]
                slot0 = SLOT0[L]
                r0 = b * R
                if stash:
                    # L1 block 0 runs from the xb0 stash (its ysb input
                    # window is overwritten by block 1's output by now)
                    src_t = xbs[0]
                    loc = lambda rr, dy: (rr + 1 + dy)
                elif L == 0:
                    src_t = xbs[b % 4]
                    loc = lambda rr, dy: (rr - r0 + 1 + dy)  # slot in xb
                else:
                    src_t = ysb
                    loc = lambda rr, dy: (rr + dy + 1)       # y1 slot

                for cp in range(R // 2):
                    rr = r0 + 2 * cp
                    pt = psc.tile([128, 512], f32, tag="cps",
                                  name=f"c{L}_{b}_{cp}")
                    for t in range(9):
                        dy, dx = t // 3 - 1, t % 3 - 1
                        off = yoff(loc(rr, dy)) + dx
                        rhs = _ap(src_t[:], off, [[PITCH, 2], [1, W]])
                        nc.tensor.matmul(
                            pt[:], wt[:, t, :], rhs,
                            start=(t == 0), stop=(t == 8),
                        )
                    nc.scalar.copy(
                        out=_ap(ysb[:], yoff(rr + slot0), [[PITCH, 2], [1, W]]),
                        in_=pt[:],
                    )
                if L == 0 and not stash and b + 2 < NB:
                    xb_load(b + 2)

            def transp_block(L, b):
                slot0 = SLOT0[L]
                r0 = b * R
                pts2 = ptp.tile([128, 1024], f16, tag="tp", name=f"tp{L}_{b}")
                for j in range(2 * R):
                    rr = r0 + j // 2
                    cs = j % 2
                    src = _ap(ysb[:], yoff(rr + slot0) + cs * 128, [[1, 128]])
                    nc.tensor.transpose(
                        pts2[:, j * 128 : (j + 1) * 128], src, id128sb[:]
                    )
                # strip layout per chunk: [yA(64) y2A(64) yB(64) y2B(64)]
                sp = stripp.tile([128, 2 * R, 256], f8, tag="strip",
                                 name=f"sp{L}_{b}")
                strip_tiles[L][b] = sp
                nc.scalar.copy(
                    out=_ap(sp[:], 0, [[256, 2 * R], [1, 64]]),
                    in_=_ap(pts2[:], 0, [[128, 2 * R], [1, 64]]),
                )
                nc.scalar.copy(
                    out=_ap(sp[:], 128, [[256, 2 * R], [1, 64]]),
                    in_=_ap(pts2[:], 64, [[128, 2 * R], [1, 64]]),
                )
                nc.vector.tensor_tensor(
                    _ap(sp[:], 64, [[128, 4 * R], [1, 64]]),
                    _ap(sp[:], 0, [[128, 4 * R], [1, 64]]),
                    _ap(sp[:], 0, [[128, 4 * R], [1, 64]]),
                    ALU.mult,
                )

            def stats_block(L, b):
                sp = strip_tiles[L].pop(b)
                for j in range(2 * R):
                    ci = b * 2 * R + j
                    lhsT = _ap(maskpm[:], ci * 64, [[32, 2], [1, 18]])
                    rhs = _ap(sp[:], j * 256, [[128, 2], [1, 128]])
                    nc.tensor.matmul(
                        stats_t[L][:], lhsT, rhs,
                        start=(scnt[L] == 0),
                        stop=(scnt[L] == NCI - 1),
                        perf_mode=DRM,
                    )
                    scnt[L] += 1

            def finalize(L):
                stats = stats_t[L]
                mean = smallp.tile([18, C], f32, tag="mean")
                e2 = smallp.tile([18, C], f32, tag="e2")
                nc.vector.tensor_scalar_mul(out=mean[:], in0=stats[:, 0:64],
                                            scalar1=rcsb[:])
                nc.vector.tensor_scalar_mul(out=e2[:], in0=stats[:, 64:128],
                                            scalar1=rcsb[:])
                var = smallp.tile([18, C], f32, tag="var")
                # mean^2 on Act, in parallel with the DVE chain
                nc.scalar.activation(out=var[:], in_=mean[:], func=ACT.Square)
                nc.vector.tensor_tensor(var[:], e2[:], var[:], ALU.subtract)
                sd = smallp.tile([18, C], f32, tag="sd")
                nc.scalar.activation(out=sd[:], in_=var[:], func=ACT.Sqrt,
                                     bias=epsap[:], scale=1.0)
                rstd = smallp.tile([18, C], f32, tag="rstd")
                nc.vector.reciprocal(out=rstd[:], in_=sd[:])
                aa = smallp.tile([18, C], f32, tag="aa")
                nc.vector.tensor_tensor(aa[:], rstd[:], gam[L][:], ALU.mult)
                inv = smallp.tile([18, C], f32, tag="inv")
                nc.vector.reciprocal(out=inv[:], in_=aa[:])
                mprime = smallp.tile([18, C], f32, tag="mprime")
                nc.vector.tensor_tensor(mprime[:], bet[L][:], inv[:], ALU.mult)
                nc.vector.tensor_tensor(mprime[:], mprime[:], mean[:],
                                        ALU.subtract)
                ab2s, ab2o = ab_tiles[L]
                nc.vector.tensor_tensor(
                    ab2s[0:18, :], _ap(aa[:], 0, [[0, 2], [1, C]]), bdmsb[:],
                    ALU.mult,
                )
                nc.vector.tensor_tensor(
                    ab2o[0:18, :], _ap(mprime[:], 0, [[0, 2], [1, C]]), bdmsb[:],
                    ALU.mult,
                )
                return ab2s, ab2o

            STQ = {0: None}

            def norm_group(L, g, ab2s, ab2o):
                # tn = y + mprimeE (PE psum accumulate);
                # out = relu(tn) * aaE  (Act relu, DVE mult; gamma>0)
                slot0 = SLOT0[L]
                yv = _ap(ysb[:], yoff(2 * g + slot0), [[PITCH, 2], [1, W]])
                win = ms2[:, 2 * g * W : (2 * g + 2) * W]
                tnp = psc.tile([128, 512], f32, tag="cps", name=f"tn{L}_{g}")
                sEp = psc.tile([128, 512], f32, tag="cps", name=f"sE{L}_{g}")
                nc.tensor.matmul(tnp[:], ab2o[:], win, start=True, stop=False)
                nc.tensor.matmul(tnp[:], id128sb[:], yv, start=False, stop=True)
                nc.tensor.matmul(sEp[:], ab2s[:], win, start=True, stop=True)
                tr = normp.tile([128, 512], f16, tag="tr", name=f"tr{L}_{g}")
                nc.scalar.activation(out=tr[:], in_=tnp[:], func=ACT.Relu)
                if L == 0:
                    dst = yv
                else:
                    gl = g % GS
                    if gl == 0:
                        STQ[0] = outp.tile([128, GS * 512], f16, tag="st",
                                           name=f"st{g // GS}")
                    dst = STQ[0][:, gl * 512 : (gl + 1) * 512]
                nc.vector.tensor_tensor(dst, tr[:], sEp[:], ALU.mult)
                if L == 1 and g % GS == GS - 1:
                    st = STQ[0]
                    gb = g // GS
                    eng = nc.sync if gb % 2 == 0 else nc.scalar
                    eng.dma_start(
                        out=bass.AP(tensor=out[:].tensor,
                                    offset=gb * 2 * GS * W,
                                    ap=[[HW, 64], [1, 2 * GS * W]]),
                        in_=st[0:64, :],
                    )
                    eng2 = nc.scalar if gb % 2 == 0 else nc.sync
                    eng2.dma_start(
                        out=bass.AP(tensor=out[:].tensor,
                                    offset=HW2 + gb * 2 * GS * W,
                                    ap=[[HW, 64], [1, 2 * GS * W]]),
                        in_=st[64:128, :],
                    )

            # ================= layer 0: conv + stats =================
            conv_block(0, 0)
            build_masks()
            if NB > 1:
                conv_block(0, 1)
            transp_block(0, 0)
            for b in range(2, NB):
                conv_block(0, b)
                transp_block(0, b - 1)
                stats_block(0, b - 2)
            transp_block(0, NB - 1)
            if NB > 1:
                stats_block(0, NB - 2)
            stats_block(0, NB - 1)
            ab2s0, ab2o0 = finalize(0)

            # ===== fused: layer-0 normalize + layer-1 conv/stats =====
            # L1 conv block order [1..NB-1, 0]: block 0 needs the B-half
            # top halo (= normalized A row HH-1, ready only after the last
            # norm group), block NB-1 needs the A-half bottom halo (= B row
            # 0, ready after group 0).
            seq = []

            def push_l1(bb):
                conv_block(1, bb, stash=(bb == 0))
                seq.append(bb)
                if len(seq) >= 2:
                    transp_block(1, seq[-2])
                if len(seq) >= 3:
                    stats_block(1, seq[-3])

            norm_group(0, 0, ab2s0, ab2o0)
            # A-half bottom halo: slot HH+1 <- normalized B row 0 (slot 1)
            nc.sync.dma_start(
                out=_ap(ysb[0:64, :], yoff(HH + 1), [[1, W]]),
                in_=_ap(ysb[64:128, :], yoff(1), [[1, W]]),
            )
            for g in range(1, HH // 2):
                norm_group(0, g, ab2s0, ab2o0)
                if g == 2:
                    # stash L1-block-0's input window (y1n rows 0..4, both
                    # halves) into xb0 before block 1's output clobbers it
                    nc.vector.memset(_ap(xbs[0][0:64, :], yoff(0), [[1, W]]),
                                     0.0)
                    nc.scalar.copy(
                        out=_ap(xbs[0][:], yoff(1), [[PITCH, R + 1], [1, W]]),
                        in_=_ap(ysb[:], yoff(1), [[PITCH, R + 1], [1, W]]),
                    )
                if g >= 4 and g % 2 == 0:
                    bb = g // 2 - 1
                    if 1 <= bb <= NB - 2:
                        push_l1(bb)
            if NB > 2:
                push_l1(NB - 1)
            # B-half top halo for the stashed block 0:
            # xb0 B slot 0 <- normalized A row HH-1 (slot HH)
            nc.sync.dma_start(
                out=_ap(xbs[0][64:128, :], yoff(0), [[1, W]]),
                in_=_ap(ysb[0:64, :], yoff(HH), [[1, W]]),
            )
            push_l1(0)
            if NB == 2:
                push_l1(1)
            # drain the transpose/stats pipeline tail
            transp_block(1, seq[-1])
            stats_block(1, seq[-2])
            stats_block(1, seq[-1])
            ab2s1, ab2o1 = finalize(1)

            # ================= layer 1 normalize + store =================
            for g in range(HH // 2):
                norm_group(1, g, ab2s1, ab2o1)

    return nc


MAXW = 1


def _split_multi_waits(nc):
    """The installed walrus rejects instructions with >MAXW sync waits; hoist
    excess waits onto preceding same-engine nops."""
    nsplit = 0
    for fn in nc.m.functions:
        for blk in fn.blocks:
            insts = list(blk.instructions)
            out = []
            for inst in insts:
                si = inst.sync_info
                waits = list(si.on_wait) if (si and si.on_wait) else []
                if len(waits) > MAXW:
                    for i in range(0, len(waits) - MAXW, MAXW):
                        nop = mybir.InstNoOp(
                            name=f"WSPLIT-{nsplit}", ins=[], outs=[]
                        )
                        nsplit += 1
                        nop.engine = inst.engine
                        nop.sync_info = mybir.SyncInfo(
                            on_wait=waits[i : i + MAXW], on_update=[]
                        )
                        out.append(nop)
                    si.on_wait = waits[len(waits) - MAXW :]
                out.append(inst)
            if len(out) != len(insts):
                while len(blk.instructions):
                    blk.instructions.pop()
                for inst in out:
                    blk.instructions.append(inst)
    return nsplit


def build_nc(H=256, split_waits=True):
    _install_tile_patch()
    nc = bass.Bass()
    emit(nc, H)
    if split_waits:
        n = _split_multi_waits(nc)
        if n:
            print(f"kernel: split {n} multi-wait instructions")
    return nc


# ---------------------------------------------------------------------------
# host-side input prep
# ---------------------------------------------------------------------------
def prep_core_inputs(x_img, ids_img, w0, g0v, b0v, w1, g1v, b1v, H=256):
    """x_img [C,H,W] f32, ids_img [H,W] int -> input map for one core."""
    seg = np.where(ids_img < 0, 8, ids_img).astype(np.int64)

    m = {}
    m["xh"] = np.ascontiguousarray(x_img.reshape(C, H * W).astype(np.float16))
    m["idsf"] = np.ascontiguousarray(ids_img.reshape(H * W).astype(np.float16))
    cnt = np.bincount(seg.reshape(-1), minlength=9)[:9]
    rc9 = (1.0 / np.maximum(cnt, 1)).astype(np.float32)
    rc9[8] = 0.0  # background: forces mean=var=0 -> rstd=1/sqrt(eps)
    rc = np.concatenate([rc9, rc9])
    m["rcnt"] = rc.reshape(18, 1).astype(np.float32)
    kv9 = np.array([0, 1, 2, 3, 4, 5, 6, 7, -1], np.float32)
    m["kvec"] = np.concatenate([kv9, kv9]).reshape(18, 1)

    for name, wmat in (("w0d", w0), ("w1d", w1)):
        wd = np.zeros((9, 128, 128), np.float16)
        for t in range(9):
            dy, dx = t // 3, t % 3
            lhsT = wmat[:, :, dy, dx].T.astype(np.float16)  # [cin, cout]
            wd[t, 0:64, 0:64] = lhsT
            wd[t, 64:128, 64:128] = lhsT
        m[name] = np.ascontiguousarray(wd.transpose(1, 0, 2))  # [ci, t, co]

    m["id128"] = np.eye(128, dtype=np.float16)
    bdmask = np.zeros((18, 128), np.float16)
    bdmask[0:9, 0:64] = 1.0
    bdmask[9:18, 64:128] = 1.0
    m["bdm"] = bdmask
    for nmg, nmb, gv, bv in (("g18a", "b18a", g0v, b0v), ("g18b", "b18b", g1v, b1v)):
        g9 = np.broadcast_to(np.asarray(gv, np.float32), (9, C)).copy()
        b9 = np.broadcast_to(np.asarray(bv, np.float32), (9, C)).copy()
        g9[8, :] = np.sqrt(EPS)   # background row: aa = rstd*sqrt(eps) = 1
        b9[8, :] = 0.0
        m[nmg] = np.concatenate([g9, g9], 0).astype(np.float32)
        m[nmb] = np.concatenate([b9, b9], 0).astype(np.float32)
    return m


LAST_RESULT = None


def kernel(features, ins_indices_batch, w0, g0, b0, w1, g1, b1):
    global LAST_RESULT
    _install_ntff_shim()
    from concourse.bass_utils import run_bass_kernel_spmd
    from concourse import bass2jax as _b2j
    import traceback as _tb

    _b2j.install_neuronx_cc_hook()
    import libneuronxla as _lnx

    if not getattr(_lnx, "_ant_dbg_wrapped", False):
        _orig = _lnx.neuronx_cc

        def _dbg(*a, **k):
            try:
                return _orig(*a, **k)
            except BaseException:
                _tb.print_exc()
                raise

        _lnx.neuronx_cc = _dbg
        _lnx._ant_dbg_wrapped = True

    x = np.asarray(features, np.float32)
    ids = np.asarray(ins_indices_batch).astype(np.int64)
    w0 = np.asarray(w0, np.float32)
    w1 = np.asarray(w1, np.float32)
    N = x.shape[0]
    H = x.shape[2]

    nc = build_nc(H)
    in_maps = [
        prep_core_inputs(x[i], ids[i], w0, g0, b0, w1, g1, b1, H) for i in range(N)
    ]
    trace = bool(int(os.environ.get("BASS_KERNEL_TRACE", "0")))
    res = run_bass_kernel_spmd(nc, in_maps, list(range(N)), trace=trace)
    LAST_RESULT = res
    outs = [
        np.asarray(res.results[i]["out"], np.float32).reshape(C, H, W)
        for i in range(N)
    ]
    return np.stack(outs, 0)


# revision 33
# speedup vs baseline: 1.1553x; 1.0102x over previous
"""Trainium2 Bass kernel for nn_DensePoseV1ConvXGNInsHead:
2x (conv3x3 64->64 -> per-instance BN -> ReLU) on [8,64,256,256],
data-parallel one image per NeuronCore across 8 cores.

Structure (per core / image; A = rows 0:128 on partitions 0:64,
B = rows 128:256 on partitions 64:128):
 - conv3x3 as 9 shifted fp16 matmuls per 2-row chunk, block-diagonal
   [A|B] 128-partition weights, PSUM accumulation.
 - per-(image,instance) BN stats via PE transposes + fp8e4 DoubleRow mask
   matmuls (pair = A/B half) accumulating [18, s1|s2] in one PSUM bank;
   finalize entirely on partitions 0:18 (background handled by a host-side
   sqrt(eps) gamma row and zero inv-count).
 - normalize: tn = y + mprimeE accumulated on the PE (mask-expansion matmul
   + identity matmul into one PSUM bank); out = relu(tn) * aaE with Act
   relu + DVE multiply (gamma > 0 assumed, true for BN in this model).

Self-contained: only imports the system concourse stack from /opt/trn_rl_repo.
"""
import os
import sys
import types

sys.path.insert(0, "/opt/trn_rl_repo")

import numpy as np

import concourse.bass as bass
import concourse.tile as tile
from concourse import mybir
from concourse.vector_clock import ScopedClock

f16 = mybir.dt.float16
f32 = mybir.dt.float32
f8 = mybir.dt.float8e4
ALU = mybir.AluOpType
ACT = mybir.ActivationFunctionType
DRM = mybir.MatmulPerfMode.DoubleRow

C = 64          # channels
W = 256         # image width
PITCH = 272     # padded row pitch (16 left pad + 256 data; borrows next pad)
LP = 16         # left pad elements
R = 4           # conv rows per block (per half)
GS = 4          # norm groups per output store tile (8 rows)
EPS = 1e-5

# ---------------------------------------------------------------------------
# walrus workaround: split the Tile exit-drain's sem waits (installed walrus
# rejects instructions with >2 sync waits)
# ---------------------------------------------------------------------------
_patched = False


def _install_tile_patch():
    global _patched
    if _patched:
        return
    _patched = True

    def _drain_and_barrier(self, tick_clock, wait_clock):
        nc = self.nc
        drain_inst = nc.sync.drain()
        wait_clock.add_sem_waits(
            drain_inst.ins, ScopedClock({None: tick_clock.global_clock})
        )
        si = drain_inst.ins.sync_info
        waits = list(si.on_wait or [])
        if len(waits) > 1:
            si.on_wait = waits[:1]
            for i in range(1, len(waits)):
                nop = nc.sync.nop()
                nop.ins.sync_info = mybir.SyncInfo(
                    on_wait=waits[i : i + 1], on_update=[]
                )
        nc.all_engine_barrier()
        popped = nc._tile_sem_poison_stack.pop()
        assert popped is self._sem_poison
        nc.clear_and_free_semaphores(list(self.sems.allocated().values()))
        nc.all_engine_barrier()

    tile.TileContext._drain_and_barrier = _drain_and_barrier


# ---------------------------------------------------------------------------
# NTFF profiling shim (antenv.axon_hooks is absent in this image)
# ---------------------------------------------------------------------------
def _install_ntff_shim():
    if "antenv.axon_hooks" in sys.modules:
        return
    mod = types.ModuleType("antenv.axon_hooks")
    state = {"hook": None}
    mod.set_axon_ntff_profile_hook = lambda h: state.__setitem__("hook", h)
    mod.get_axon_ntff_profile_hook = lambda: state["hook"]
    sys.modules["antenv.axon_hooks"] = mod
    try:
        import antenv

        antenv.axon_hooks = mod
    except ImportError:
        pass
    try:
        from trn_agent_boot.trn_boot import _ntff_profile_via_ctypes

        h = _ntff_profile_via_ctypes("/opt/axon/libaxon_pjrt.so")
        mod.set_axon_ntff_profile_hook(h)
    except Exception:
        pass


def yoff(slot):
    return slot * PITCH + LP


def _ap(base_ap, offset_elems, dims):
    """Build a sub-AP of base_ap at +offset (elements), with given free dims."""
    return bass.AP(
        tensor=base_ap.tensor,
        offset=base_ap.offset + offset_elems,
        ap=[base_ap.ap[0]] + dims,
    )


def emit(nc, H):
    """Emit the full 2-layer kernel for an HxW image (H=256 in production)."""
    HH = H // 2
    NB = HH // R            # conv blocks per layer
    NCI = HH * 2            # 128-px chunk pairs (A+B) per layer
    HW = H * W
    HW2 = HH * W
    assert HH % R == 0 and (HH // 2) % GS == 0

    xh = nc.declare_dram_parameter("xh", [C, HW], f16, isOutput=False)
    idsf = nc.declare_dram_parameter("idsf", [HW], f16, isOutput=False)
    rcnt = nc.declare_dram_parameter("rcnt", [18, 1], f32, isOutput=False)
    kvec = nc.declare_dram_parameter("kvec", [18, 1], f32, isOutput=False)
    w0d = nc.declare_dram_parameter("w0d", [128, 9, 128], f16, isOutput=False)
    w1d = nc.declare_dram_parameter("w1d", [128, 9, 128], f16, isOutput=False)
    id128 = nc.declare_dram_parameter("id128", [128, 128], f16, isOutput=False)
    g18a = nc.declare_dram_parameter("g18a", [18, C], f32, isOutput=False)
    b18a = nc.declare_dram_parameter("b18a", [18, C], f32, isOutput=False)
    g18b = nc.declare_dram_parameter("g18b", [18, C], f32, isOutput=False)
    b18b = nc.declare_dram_parameter("b18b", [18, C], f32, isOutput=False)
    bdm = nc.declare_dram_parameter("bdm", [18, 128], f16, isOutput=False)
    out = nc.declare_dram_parameter("out", [C, HW], f16, isOutput=True)

    with tile.TileContext(nc) as tc:
        import contextlib

        with contextlib.ExitStack() as ctx:
            const = ctx.enter_context(tc.tile_pool(name="const", bufs=1))
            xbp = ctx.enter_context(tc.tile_pool(name="xbp", bufs=1))
            stripp = ctx.enter_context(tc.tile_pool(name="stripp", bufs=3))
            normp = ctx.enter_context(tc.tile_pool(name="normp", bufs=3))
            outp = ctx.enter_context(tc.tile_pool(name="outp", bufs=2))
            smallp = ctx.enter_context(tc.tile_pool(name="smallp", bufs=2))
            idsmp = ctx.enter_context(tc.tile_pool(name="idsmp", bufs=2))
            psc = ctx.enter_context(tc.tile_pool(name="psc", bufs=5, space="PSUM"))
            ptp = ctx.enter_context(tc.tile_pool(name="ptp", bufs=2, space="PSUM"))
            pss = ctx.enter_context(tc.tile_pool(name="pss", bufs=1, space="PSUM"))

            # ---- persistent y buffer (pitched, slots 0..HH+1 per half)
            ysb = const.tile([128, (HH + 2) * PITCH + LP], f16)
            # zero: all left pads (incl. trailing pad), top halo A, bottom halo B
            nc.vector.memset(_ap(ysb[:], 0, [[PITCH, HH + 3], [1, LP]]), 0.0)
            nc.vector.memset(_ap(ysb[0:64, :], yoff(0), [[1, W]]), 0.0)
            nc.vector.memset(_ap(ysb[64:128, :], yoff(HH + 1), [[1, W]]), 0.0)

            xbs = []
            for i in range(4):
                xb = xbp.tile([128, (R + 2) * PITCH + LP], f16, tag=f"xb{i}")
                nc.vector.memset(_ap(xb[:], 0, [[PITCH, R + 3], [1, LP]]), 0.0)
                xbs.append(xb)

            def xb_load(b, eng=None):
                eng = eng or nc.sync
                r0 = b * R
                xb = xbs[b % 4]
                if b == 0:
                    nc.vector.memset(
                        _ap(xb[0:64, :], yoff(0), [[1, W]]), 0.0
                    )
                if b == NB - 1:
                    nc.vector.memset(
                        _ap(xb[64:128, :], yoff(R + 1), [[1, W]]), 0.0
                    )
                lo_a = r0 - 1
                s_a = 0
                if b == 0:
                    lo_a, s_a = 0, 1
                n_a = r0 + R - lo_a + 1
                eng.dma_start(
                    out=_ap(xb[0:64, :], yoff(s_a), [[PITCH, n_a], [1, W]]),
                    in_=bass.AP(
                        tensor=xh[:].tensor,
                        offset=lo_a * W,
                        ap=[[HW, 64], [W, n_a], [1, W]],
                    ),
                )
                hb_lo = HH + r0 - 1
                n_b = R + 2 if b < NB - 1 else R + 1
                eng.dma_start(
                    out=_ap(xb[64:128, :], yoff(0), [[PITCH, n_b], [1, W]]),
                    in_=bass.AP(
                        tensor=xh[:].tensor,
                        offset=hb_lo * W,
                        ap=[[HW, 64], [W, n_b], [1, W]],
                    ),
                )

            # first conv inputs + layer-0 weights first on the DMA queue;
            # w0 is loaded per-tap so block 0's matmuls start after tap 0
            wts = []
            for wd in (w0d, w1d):
                wt = const.tile([128, 9, 128], f16, tag="wt")
                wts.append(wt)
            xb_load(0)
            nc.sync.dma_start(out=wts[0][:, 0:2, :], in_=w0d[:, 0:2, :])
            if NB > 1:
                xb_load(1)
            nc.sync.dma_start(out=wts[0][:, 2:9, :], in_=w0d[:, 2:9, :])

            # ---- small constants
            id128sb = const.tile([128, 128], f16)
            nc.sync.dma_start(out=id128sb[:], in_=id128[:])
            rcsb = const.tile([18, 1], f32)
            nc.sync.dma_start(out=rcsb[:], in_=rcnt[:])
            kvecsb = const.tile([18, 1], f32)
            nc.sync.dma_start(out=kvecsb[:], in_=kvec[:])
            epsap = const.tile([18, 1], f32)
            nc.vector.memset(epsap[:], EPS)
            ktile = const.tile([128, 9], f16)
            nc.gpsimd.iota(
                ktile[:], pattern=[[1, 9]], base=0, channel_multiplier=0,
                allow_small_or_imprecise_dtypes=True,
            )
            nc.vector.memset(ktile[:, 8:9], -1.0)
            bdmsb = const.tile([18, 128], f16)
            nc.sync.dma_start(out=bdmsb[:], in_=bdm[:])
            gam = []
            bet = []
            for gg, bb in ((g18a, b18a), (g18b, b18b)):
                gt = const.tile([18, C], f32, tag="gam")
                bt = const.tile([18, C], f32, tag="bet")
                nc.sync.dma_start(out=gt[:], in_=gg[:])
                nc.sync.dma_start(out=bt[:], in_=bb[:])
                gam.append(gt)
                bet.append(bt)
            nc.sync.dma_start(out=wts[1][:], in_=w1d[:])

            # ---- ids: pixel-major [128 px, global chunks] via PE
            # transpose. The load is issued here (DMA queue position) but
            # the PE transposes are emitted after conv block 0 so they do
            # not gate the in-order PE's start on the idsq DMA.
            F = HW // 128   # elements per partition in the contiguous load
            idp2 = const.tile([128, HW // 128], f16)
            maskpm = const.tile([128, NCI, 64], f8)
            if F % 128 == 0:
                idsq = idsmp.tile([128, F], f16, tag="idsq")
                nc.sync.dma_start(
                    out=idsq[:],
                    in_=bass.AP(tensor=idsf[:].tensor, offset=0,
                                ap=[[F, 128], [1, F]]),
                )
            else:
                nc.sync.dma_start(
                    out=idp2[:],
                    in_=bass.AP(tensor=idsf[:].tensor, offset=0,
                                ap=[[1, 128], [128, F]]),
                )

            def build_masks():
                if F % 128 == 0:
                    KT = F // 128
                    for k in range(KT):
                        ptsI = psc.tile([128, 128], f16, tag="cps",
                                        name=f"idT{k}")
                        nc.tensor.transpose(
                            ptsI[:], idsq[:, 128 * k : 128 * (k + 1)],
                            id128sb[:]
                        )
                        nc.vector.tensor_copy(
                            _ap(idp2[:], k, [[KT, 128]]), ptsI[:]
                        )
                # pixel-major one-hot masks, f8, duplicated per half, padded
                # so the DoubleRow stats lhsT is [[32,2],[1,18]] (16B pair
                # stride): per ci, 64 cols: A-dup18 at 0, B-dup18 at 32
                for h in (0, 1):
                    for d in (0, 1):
                        nc.vector.tensor_tensor(
                            _ap(maskpm[:], 32 * h + 9 * d,
                                [[64, NCI], [1, 9]]),
                            _ap(idp2[:], h * NCI, [[1, NCI], [0, 9]]),
                            _ap(ktile[:], 0, [[0, NCI], [1, 9]]),
                            ALU.is_equal,
                        )

            # segment-major one-hot masks, f8, padded to 128 partitions
            # (zeros; small-partition matmul inputs stream slowly on HW)
            ms2 = const.tile([128, HW2], f8)
            nc.gpsimd.memset(ms2[:], 0.0)
            MCH = min(2048, HW2)
            for mc in range(HW2 // MCH):
                idsm = idsmp.tile([18, MCH], f16, tag="idsm", name=f"idsm{mc}")
                nc.sync.dma_start(
                    out=idsm[:],
                    in_=bass.AP(
                        tensor=idsf[:].tensor,
                        offset=mc * MCH,
                        ap=[[HW2, 2], [0, 9], [1, MCH]],
                    ),
                )
                nc.vector.tensor_scalar(
                    out=ms2[0:18, mc * MCH : (mc + 1) * MCH], in0=idsm[:],
                    scalar1=kvecsb[:], scalar2=None, op0=ALU.is_equal,
                )

            # block-diagonal f16 lhsT tiles for the expansion matmuls;
            # rows 18:128 stay zero (contraction padded to 128, matching
            # ms2). Allocated/memset up-front, off the finalize chain.
            ab_tiles = {}
            for LL in (0, 1):
                a_s = smallp.tile([128, 128], f16, tag="ab2s")
                a_o = smallp.tile([128, 128], f16, tag="ab2o")
                nc.vector.memset(a_s[:], 0.0)
                nc.vector.memset(a_o[:], 0.0)
                ab_tiles[LL] = (a_s, a_o)

            SLOT0 = {0: 1, 1: 0}     # y row r lives at slot r+SLOT0[L]
            stats_t = {}
            strip_tiles = {0: {}, 1: {}}
            scnt = {0: 0, 1: 0}
            for LL in (0, 1):
                stats_t[LL] = pss.tile([18, 128], f32, tag="stats",
                                       name=f"stats{LL}")

            def conv_block(L, b, stash=False):
                wt = wts[L]
                slot0 = SLOT0[L]
                r0 = b * R
                if stash:
                    # L1 block 0 runs from the xb0 stash (its ysb input
                    # window is overwritten by block 1's output by now)
                    src_t = xbs[0]
                    loc = lambda rr, dy: (rr + 1 + dy)
                elif L == 0:
                    src_t = xbs[b % 4]
                    loc = lambda rr, dy: (rr - r0 + 1 + dy)  # slot in xb
                else:
                    src_t = ysb
                    loc = lambda rr, dy: (rr + dy + 1)       # y1 slot

                for cp in range(R // 2):
                    rr = r0 + 2 * cp
                    pt = psc.tile([128, 512], f32, tag="cps",
                                  name=f"c{L}_{b}_{cp}")
                    for t in range(9):
                        dy, dx = t // 3 - 1, t % 3 - 1
                        off = yoff(loc(rr, dy)) + dx
                        rhs = _ap(src_t[:], off, [[PITCH, 2], [1, W]])
                        nc.tensor.matmul(
                            pt[:], wt[:, t, :], rhs,
                            start=(t == 0), stop=(t == 8),
                        )
                    nc.scalar.copy(
                        out=_ap(ysb[:], yoff(rr + slot0), [[PITCH, 2], [1, W]]),
                        in_=pt[:],
                    )
                if L == 0 and not stash and b + 2 < NB:
                    xb_load(b + 2)

            def transp_block(L, b):
                slot0 = SLOT0[L]
                r0 = b * R
                pts2 = ptp.tile([128, 1024], f16, tag="tp", name=f"tp{L}_{b}")
                for j in range(2 * R):
                    rr = r0 + j // 2
                    cs = j % 2
                    src = _ap(ysb[:], yoff(rr + slot0) + cs * 128, [[1, 128]])
                    nc.tensor.transpose(
                        pts2[:, j * 128 : (j + 1) * 128], src, id128sb[:]
                    )
                # strip layout per chunk: [yA(64) y2A(64) yB(64) y2B(64)]
                sp = stripp.tile([128, 2 * R, 256], f8, tag="strip",
                                 name=f"sp{L}_{b}")
                strip_tiles[L][b] = sp
                nc.scalar.copy(
                    out=_ap(sp[:], 0, [[256, 2 * R], [1, 64]]),
                    in_=_ap(pts2[:], 0, [[128, 2 * R], [1, 64]]),
                )
                nc.scalar.copy(
                    out=_ap(sp[:], 128, [[256, 2 * R], [1, 64]]),
                    in_=_ap(pts2[:], 64, [[128, 2 * R], [1, 64]]),
                )
                nc.vector.tensor_tensor(
                    _ap(sp[:], 64, [[128, 4 * R], [1, 64]]),
                    _ap(sp[:], 0, [[128, 4 * R], [1, 64]]),
                    _ap(sp[:], 0, [[128, 4 * R], [1, 64]]),
                    ALU.mult,
                )

            def stats_block(L, b):
                sp = strip_tiles[L].pop(b)
                for j in range(2 * R):
                    ci = b * 2 * R + j
                    lhsT = _ap(maskpm[:], ci * 64, [[32, 2], [1, 18]])
                    rhs = _ap(sp[:], j * 256, [[128, 2], [1, 128]])
                    nc.tensor.matmul(
                        stats_t[L][:], lhsT, rhs,
                        start=(scnt[L] == 0),
                        stop=(scnt[L] == NCI - 1),
                        perf_mode=DRM,
                    )
                    scnt[L] += 1

            def finalize(L):
                stats = stats_t[L]
                mean = smallp.tile([18, C], f32, tag="mean")
                e2 = smallp.tile([18, C], f32, tag="e2")
                nc.vector.tensor_scalar_mul(out=mean[:], in0=stats[:, 0:64],
                                            scalar1=rcsb[:])
                nc.vector.tensor_scalar_mul(out=e2[:], in0=stats[:, 64:128],
                                            scalar1=rcsb[:])
                var = smallp.tile([18, C], f32, tag="var")
                # mean^2 on Act, in parallel with the DVE chain
                nc.scalar.activation(out=var[:], in_=mean[:], func=ACT.Square)
                nc.vector.tensor_tensor(var[:], e2[:], var[:], ALU.subtract)
                sd = smallp.tile([18, C], f32, tag="sd")
                nc.scalar.activation(out=sd[:], in_=var[:], func=ACT.Sqrt,
                                     bias=epsap[:], scale=1.0)
                rstd = smallp.tile([18, C], f32, tag="rstd")
                nc.vector.reciprocal(out=rstd[:], in_=sd[:])
                aa = smallp.tile([18, C], f32, tag="aa")
                nc.vector.tensor_tensor(aa[:], rstd[:], gam[L][:], ALU.mult)
                inv = smallp.tile([18, C], f32, tag="inv")
                nc.vector.reciprocal(out=inv[:], in_=aa[:])
                mprime = smallp.tile([18, C], f32, tag="mprime")
                nc.vector.tensor_tensor(mprime[:], bet[L][:], inv[:], ALU.mult)
                nc.vector.tensor_tensor(mprime[:], mprime[:], mean[:],
                                        ALU.subtract)
                ab2s, ab2o = ab_tiles[L]
                nc.vector.tensor_tensor(
                    ab2s[0:18, :], _ap(aa[:], 0, [[0, 2], [1, C]]), bdmsb[:],
                    ALU.mult,
                )
                nc.vector.tensor_tensor(
                    ab2o[0:18, :], _ap(mprime[:], 0, [[0, 2], [1, C]]), bdmsb[:],
                    ALU.mult,
                )
                return ab2s, ab2o

            STQ = {0: None}

            def norm_group(L, g, ab2s, ab2o):
                # tn = y + mprimeE (PE psum accumulate);
                # out = relu(tn) * aaE  (Act relu, DVE mult; gamma>0)
                slot0 = SLOT0[L]
                yv = _ap(ysb[:], yoff(2 * g + slot0), [[PITCH, 2], [1, W]])
                win = ms2[:, 2 * g * W : (2 * g + 2) * W]
                tnp = psc.tile([128, 512], f32, tag="cps", name=f"tn{L}_{g}")
                sEp = psc.tile([128, 512], f32, tag="cps", name=f"sE{L}_{g}")
                nc.tensor.matmul(tnp[:], ab2o[:], win, start=True, stop=False)
                nc.tensor.matmul(tnp[:], id128sb[:], yv, start=False, stop=True)
                nc.tensor.matmul(sEp[:], ab2s[:], win, start=True, stop=True)
                tr = normp.tile([128, 512], f16, tag="tr", name=f"tr{L}_{g}")
                nc.scalar.activation(out=tr[:], in_=tnp[:], func=ACT.Relu)
                if L == 0:
                    dst = yv
                else:
                    gl = g % GS
                    if gl == 0:
                        STQ[0] = outp.tile([128, GS * 512], f16, tag="st",
                                           name=f"st{g // GS}")
                    dst = STQ[0][:, gl * 512 : (gl + 1) * 512]
                nc.vector.tensor_tensor(dst, tr[:], sEp[:], ALU.mult)
                if L == 1:
                    last_tile = (g // GS) == (HH // 2) // GS - 1
                    if last_tile:
                        # stream the final tile per-group so the kernel tail
                        # doesn't wait on one big store after the last group
                        stt = STQ[0]
                        gl = g % GS
                        eng = nc.sync if g % 2 == 0 else nc.scalar
                        eng.dma_start(
                            out=bass.AP(tensor=out[:].tensor,
                                        offset=2 * g * W,
                                        ap=[[HW, 64], [1, 2 * W]]),
                            in_=stt[0:64, gl * 512 : (gl + 1) * 512],
                        )
                        eng2 = nc.scalar if g % 2 == 0 else nc.sync
                        eng2.dma_start(
                            out=bass.AP(tensor=out[:].tensor,
                                        offset=HW2 + 2 * g * W,
                                        ap=[[HW, 64], [1, 2 * W]]),
                            in_=stt[64:128, gl * 512 : (gl + 1) * 512],
                        )
                    elif g % GS == GS - 1:
                        st = STQ[0]
                        gb = g // GS
                        eng = nc.sync if gb % 2 == 0 else nc.scalar
                        eng.dma_start(
                            out=bass.AP(tensor=out[:].tensor,
                                        offset=gb * 2 * GS * W,
                                        ap=[[HW, 64], [1, 2 * GS * W]]),
                            in_=st[0:64, :],
                        )
                        eng2 = nc.scalar if gb % 2 == 0 else nc.sync
                        eng2.dma_start(
                            out=bass.AP(tensor=out[:].tensor,
                                        offset=HW2 + gb * 2 * GS * W,
                                        ap=[[HW, 64], [1, 2 * GS * W]]),
                            in_=st[64:128, :],
                        )

            # ================= layer 0: conv + stats =================
            conv_block(0, 0)
            if NB > 1:
                conv_block(0, 1)
            build_masks()
            transp_block(0, 0)
            for b in range(2, NB):
                conv_block(0, b)
                transp_block(0, b - 1)
                stats_block(0, b - 2)
            transp_block(0, NB - 1)
            if NB > 1:
                stats_block(0, NB - 2)
            stats_block(0, NB - 1)
            ab2s0, ab2o0 = finalize(0)

            # ===== fused: layer-0 normalize + layer-1 conv/stats =====
            # L1 conv block order [1..NB-1, 0]: block 0 needs the B-half
            # top halo (= normalized A row HH-1, ready only after the last
            # norm group), block NB-1 needs the A-half bottom halo (= B row
            # 0, ready after group 0).
            seq = []

            def push_l1(bb):
                conv_block(1, bb, stash=(bb == 0))
                seq.append(bb)
                if len(seq) >= 2:
                    transp_block(1, seq[-2])
                if len(seq) >= 3:
                    stats_block(1, seq[-3])

            norm_group(0, 0, ab2s0, ab2o0)
            # A-half bottom halo: slot HH+1 <- normalized B row 0 (slot 1)
            nc.sync.dma_start(
                out=_ap(ysb[0:64, :], yoff(HH + 1), [[1, W]]),
                in_=_ap(ysb[64:128, :], yoff(1), [[1, W]]),
            )
            for g in range(1, HH // 2):
                norm_group(0, g, ab2s0, ab2o0)
                if g == 2:
                    # stash L1-block-0's input window (y1n rows 0..4, both
                    # halves) into xb0 before block 1's output clobbers it
                    nc.vector.memset(_ap(xbs[0][0:64, :], yoff(0), [[1, W]]),
                                     0.0)
                    nc.scalar.copy(
                        out=_ap(xbs[0][:], yoff(1), [[PITCH, R + 1], [1, W]]),
                        in_=_ap(ysb[:], yoff(1), [[PITCH, R + 1], [1, W]]),
                    )
                if g >= 4 and g % 2 == 0:
                    bb = g // 2 - 1
                    if 1 <= bb <= NB - 2:
                        push_l1(bb)
            if NB > 2:
                push_l1(NB - 1)
            # B-half top halo for the stashed block 0:
            # xb0 B slot 0 <- normalized A row HH-1 (slot HH)
            nc.sync.dma_start(
                out=_ap(xbs[0][64:128, :], yoff(0), [[1, W]]),
                in_=_ap(ysb[0:64, :], yoff(HH), [[1, W]]),
            )
            push_l1(0)
            if NB == 2:
                push_l1(1)
            # drain the transpose/stats pipeline tail
            transp_block(1, seq[-1])
            stats_block(1, seq[-2])
            stats_block(1, seq[-1])
            ab2s1, ab2o1 = finalize(1)

            # ================= layer 1 normalize + store =================
            for g in range(HH // 2):
                norm_group(1, g, ab2s1, ab2o1)

    return nc


MAXW = 1


def _split_multi_waits(nc):
    """The installed walrus rejects instructions with >MAXW sync waits; hoist
    excess waits onto preceding same-engine nops."""
    nsplit = 0
    for fn in nc.m.functions:
        for blk in fn.blocks:
            insts = list(blk.instructions)
            out = []
            for inst in insts:
                si = inst.sync_info
                waits = list(si.on_wait) if (si and si.on_wait) else []
                if len(waits) > MAXW:
                    for i in range(0, len(waits) - MAXW, MAXW):
                        nop = mybir.InstNoOp(
                            name=f"WSPLIT-{nsplit}", ins=[], outs=[]
                        )
                        nsplit += 1
                        nop.engine = inst.engine
                        nop.sync_info = mybir.SyncInfo(
                            on_wait=waits[i : i + MAXW], on_update=[]
                        )
                        out.append(nop)
                    si.on_wait = waits[len(waits) - MAXW :]
                out.append(inst)
            if len(out) != len(insts):
                while len(blk.instructions):
                    blk.instructions.pop()
                for inst in out:
                    blk.instructions.append(inst)
    return nsplit


def build_nc(H=256, split_waits=True):
    _install_tile_patch()
    nc = bass.Bass()
    emit(nc, H)
    if split_waits:
        n = _split_multi_waits(nc)
        if n:
            print(f"kernel: split {n} multi-wait instructions")
    return nc


# ---------------------------------------------------------------------------
# host-side input prep
# ---------------------------------------------------------------------------
def prep_core_inputs(x_img, ids_img, w0, g0v, b0v, w1, g1v, b1v, H=256):
    """x_img [C,H,W] f32, ids_img [H,W] int -> input map for one core."""
    seg = np.where(ids_img < 0, 8, ids_img).astype(np.int64)

    m = {}
    m["xh"] = np.ascontiguousarray(x_img.reshape(C, H * W).astype(np.float16))
    m["idsf"] = np.ascontiguousarray(ids_img.reshape(H * W).astype(np.float16))
    cnt = np.bincount(seg.reshape(-1), minlength=9)[:9]
    rc9 = (1.0 / np.maximum(cnt, 1)).astype(np.float32)
    rc9[8] = 0.0  # background: forces mean=var=0 -> rstd=1/sqrt(eps)
    rc = np.concatenate([rc9, rc9])
    m["rcnt"] = rc.reshape(18, 1).astype(np.float32)
    kv9 = np.array([0, 1, 2, 3, 4, 5, 6, 7, -1], np.float32)
    m["kvec"] = np.concatenate([kv9, kv9]).reshape(18, 1)

    for name, wmat in (("w0d", w0), ("w1d", w1)):
        wd = np.zeros((9, 128, 128), np.float16)
        for t in range(9):
            dy, dx = t // 3, t % 3
            lhsT = wmat[:, :, dy, dx].T.astype(np.float16)  # [cin, cout]
            wd[t, 0:64, 0:64] = lhsT
            wd[t, 64:128, 64:128] = lhsT
        m[name] = np.ascontiguousarray(wd.transpose(1, 0, 2))  # [ci, t, co]

    m["id128"] = np.eye(128, dtype=np.float16)
    bdmask = np.zeros((18, 128), np.float16)
    bdmask[0:9, 0:64] = 1.0
    bdmask[9:18, 64:128] = 1.0
    m["bdm"] = bdmask
    for nmg, nmb, gv, bv in (("g18a", "b18a", g0v, b0v), ("g18b", "b18b", g1v, b1v)):
        g9 = np.broadcast_to(np.asarray(gv, np.float32), (9, C)).copy()
        b9 = np.broadcast_to(np.asarray(bv, np.float32), (9, C)).copy()
        g9[8, :] = np.sqrt(EPS)   # background row: aa = rstd*sqrt(eps) = 1
        b9[8, :] = 0.0
        m[nmg] = np.concatenate([g9, g9], 0).astype(np.float32)
        m[nmb] = np.concatenate([b9, b9], 0).astype(np.float32)
    return m


LAST_RESULT = None


def kernel(features, ins_indices_batch, w0, g0, b0, w1, g1, b1):
    global LAST_RESULT
    _install_ntff_shim()
    from concourse.bass_utils import run_bass_kernel_spmd
    from concourse import bass2jax as _b2j
    import traceback as _tb

    _b2j.install_neuronx_cc_hook()
    import libneuronxla as _lnx

    if not getattr(_lnx, "_ant_dbg_wrapped", False):
        _orig = _lnx.neuronx_cc

        def _dbg(*a, **k):
            try:
                return _orig(*a, **k)
            except BaseException:
                _tb.print_exc()
                raise

        _lnx.neuronx_cc = _dbg
        _lnx._ant_dbg_wrapped = True

    x = np.asarray(features, np.float32)
    ids = np.asarray(ins_indices_batch).astype(np.int64)
    w0 = np.asarray(w0, np.float32)
    w1 = np.asarray(w1, np.float32)
    N = x.shape[0]
    H = x.shape[2]

    nc = build_nc(H)
    in_maps = [
        prep_core_inputs(x[i], ids[i], w0, g0, b0, w1, g1, b1, H) for i in range(N)
    ]
    trace = bool(int(os.environ.get("BASS_KERNEL_TRACE", "0")))
    res = run_bass_kernel_spmd(nc, in_maps, list(range(N)), trace=trace)
    LAST_RESULT = res
    outs = [
        np.asarray(res.results[i]["out"], np.float32).reshape(C, H, W)
        for i in range(N)
    ]
    return np.stack(outs, 0)
